# revision 50
# baseline (speedup 1.0000x reference)
"""ExclusiveSelfAttention TRN2 kernel: head-sharded tensor parallel over 8 NeuronCores.

Sharding: 16 heads / 8 cores = 2 heads (128 channels) per core.
Each core computes q/k/v projections for its 2 heads (full sequence),
attention + per-position Gram-Schmidt exclusion (head-local), and a
partial output projection (contraction over its 128 channels).
The host sums the 8 partials, divides by 32 and adds the output bias.

Precision: projections run as fp8e4m3 DoubleRow matmuls with 3 residual
terms (w_hi x_hi + w_lo x_hi + w_hi x_lo), w pre-scaled by 32 on the host
so w and its fp8 residual stay in fp8's dynamic range; the 32x scale rides
through scores (folded into the exp scale) and v (cancels in softmax
normalization / exclusion) and is divided out on the host. Everything
else is bf16 with fp32 PSUM accumulation (fp8 attention was tried and
fails the 2e-2 gate: fp8's 3-bit mantissa on softmax weights alone costs
~2e-2). Measured rel err ~6.8e-3.

Schedule: batch-0 projection first; batch-1 projection chunks are
injected into phase (0,0)'s jt loop, which is exp(ACT)-paced and leaves
PE idle slots. PV for i2=0 runs inside the jt loop one j-tile behind
scores; i2=1 PV, exclusion, and the output projection trail each phase
and overlap the next phase's scores.
"""

import sys

if '/opt/trn_rl_repo' not in sys.path:
    sys.path.insert(0, '/opt/trn_rl_repo')

import numpy as np
import ml_dtypes

import concourse.bass as bass
import concourse.mybir as mybir
import concourse.tile as tile
from concourse.bass_utils import run_bass_kernel_spmd

F32 = mybir.dt.float32
BF16 = mybir.dt.bfloat16
FP8 = mybir.dt.float8e4
AF = mybir.ActivationFunctionType
ALU = mybir.AluOpType
DR = mybir.MatmulPerfMode.DoubleRow

B, S, D = 2, 2048, 1024
BS = B * S                    # 4096 combined (b, s) rows
HD = 64                       # head dim
E_LOC = 128                   # channels per core (2 heads)
N_CORES = 8
W_SCALE = 32.0                # host multiplies w (and biases) by this
EPS_SCALED = 1e-8 * W_SCALE * W_SCALE
EXP_SCALE = 0.125 / (W_SCALE * W_SCALE)    # 1/8 true scale on 1024x scores

_ENGINE_TO_NC = {"PE": "tensor", "DVE": "vector", "Activation": "scalar",
                 "Pool": "gpsimd", "SP": "sync"}


def _make_nop(nc, engine):
    eng = getattr(nc, _ENGINE_TO_NC[str(engine).split(".")[-1]])
    r = eng.nop(nofuse=True, hint="waitsplit")
    ins = r.ins if hasattr(r, "ins") else r
    for blk in nc.main_func.blocks:
        insns = blk.instructions
        for i, x in enumerate(insns):
            if x.name == ins.name:
                del insns[i]
                blk.instructions = insns
                return ins
    raise RuntimeError("freshly created nop not found")


def split_waits(nc, limit=1):
    """Walrus codegen only encodes one sync-wait per instruction here; move
    excess waits onto preceding same-engine NOPs (same-engine program order
    makes this semantics-preserving)."""
    for blk in nc.main_func.blocks:
        ins_list = blk.instructions
        out, changed = [], False
        for ins in ins_list:
            si = ins.sync_info
            if si is not None and len(si.on_wait) > limit:
                waits = list(si.on_wait)
                extra, keep = waits[:-limit], waits[-limit:]
                for w in extra:
                    nop = _make_nop(nc, ins.engine)
                    nop.sync_info = mybir.SyncInfo(on_wait=[w], on_update=[])
                    out.append(nop)
                ins.sync_info = mybir.SyncInfo(on_wait=keep, on_update=list(si.on_update))
                changed = True
            out.append(ins)
        if changed:
            blk.instructions = out


def build_program():
    import os
    taps = os.environ.get("KTAPS") == "1"
    nc = bass.Bass()

    xh_d = nc.declare_dram_parameter("xh", [D, BS], FP8, isOutput=False)
    xl_d = nc.declare_dram_parameter("xl", [D, BS], FP8, isOutput=False)
    w_d = {}
    for name in ("q", "k", "v"):
        w_d[name] = (nc.declare_dram_parameter(f"w{name}h", [D, E_LOC], FP8, isOutput=False),
                     nc.declare_dram_parameter(f"w{name}l", [D, E_LOC], FP8, isOutput=False))
    bq_d = nc.declare_dram_parameter("bq", [E_LOC], F32, isOutput=False)
    bk_d = nc.declare_dram_parameter("bk", [E_LOC], F32, isOutput=False)
    bv_d = nc.declare_dram_parameter("bv", [E_LOC], F32, isOutput=False)
    woT_d = nc.declare_dram_parameter("woT", [E_LOC, D], BF16, isOutput=False)
    part_d = nc.declare_dram_parameter("partial", [BS, D], BF16, isOutput=True)

    with tile.TileContext(nc) as tc:
        import contextlib
        with contextlib.ExitStack() as ctx:
            const = ctx.enter_context(tc.tile_pool(name="const", bufs=1))
            xt_pool = ctx.enter_context(tc.tile_pool(name="xt", bufs=2))
            persist = ctx.enter_context(tc.tile_pool(name="persist", bufs=1))
            et_pool = ctx.enter_context(tc.tile_pool(name="et", bufs=46))
            vn_pool = ctx.enter_context(tc.tile_pool(name="vn", bufs=32))
            sb_x = ctx.enter_context(tc.tile_pool(name="sb_x", bufs=2))
            sb_s = ctx.enter_context(tc.tile_pool(name="sb_s", bufs=2))
            out_stage = ctx.enter_context(tc.tile_pool(name="ostg", bufs=8))
            dram = ctx.enter_context(tc.tile_pool(name="dram", bufs=1, space="DRAM"))
            ps_scA = ctx.enter_context(tc.tile_pool(name="ps_scA", bufs=1, space="PSUM"))
            ps_scB = ctx.enter_context(tc.tile_pool(name="ps_scB", bufs=1, space="PSUM"))
            ps_pv = ctx.enter_context(tc.tile_pool(name="ps_pv", bufs=2, space="PSUM"))
            ps_x = ctx.enter_context(tc.tile_pool(name="ps_x", bufs=2, space="PSUM"))

            # ---- constants / weights ----
            wsb = {}
            for name in ("q", "k", "v"):
                pair = []
                for hl, wd in zip("hl", w_d[name]):
                    t = const.tile([128, 8, E_LOC], FP8, tag=f"w{name}{hl}")
                    nc.sync.dma_start(out=t, in_=wd[:, :].rearrange(
                        "(kt p) e -> p kt e", kt=8))
                    pair.append(t)
                wsb[name] = pair
            bsb = {}
            for name, bd in (("q", bq_d), ("k", bk_d), ("v", bv_d)):
                t = const.tile([128, 1], F32, tag=f"b{name}")
                nc.sync.dma_start(out=t, in_=bd[:].rearrange("(p one) -> p one", one=1))
                bsb[name] = t
            ones64 = const.tile([64, 1], BF16, tag="ones64")
            nc.vector.memset(ones64, 1.0)
            # vv reducer: h0 sum -> out partition 0, h1 sum -> partition 32
            vvw = const.tile([128, 33], BF16, tag="vvw")
            nc.vector.memset(vvw, 0.0)
            nc.vector.memset(vvw[0:64, 0:1], 1.0)
            nc.vector.memset(vvw[64:128, 32:33], 1.0)
            ones1 = const.tile([1, 64], BF16, tag="ones1")
            nc.vector.memset(ones1, 1.0)
            ones_row = const.tile([128, 32], BF16, tag="ones_row")
            nc.vector.memset(ones_row, 1.0)
            wo_sb = const.tile([128, D], BF16, tag="wo")

            # ---- persistent activations ----
            qT = persist.tile([128, BS], BF16, tag="qT")       # [e_loc, b*s]
            kT = persist.tile([128, BS], BF16, tag="kT")
            vstk = persist.tile([128, BS], BF16, tag="vstk")   # stacked heads, 32x
            o_fT = {(b, ih): persist.tile([128, 1024], BF16, tag=f"ofT{b}{ih}",
                                          name=f"ofT{b}{ih}")
                    for b in range(B) for ih in range(2)}

            from concourse.tile import add_dep_helper
            vdram = dram.tile([144, BS], BF16, tag="vdram")

            def _row_ap(r):
                return vdram[r:r + 1, :].rearrange("one (p f) -> (one p) f", p=128)

            vdw_const = [nc.gpsimd.dma_start(out=_row_ap(64), in_=ones_row),
                         nc.gpsimd.dma_start(out=_row_ap(129), in_=ones_row)]
            vdw_const += [nc.gpsimd.dma_start(out=_row_ap(130 + pr), in_=ones_row)
                          for pr in range(14)]
            vdw_b = {}
            vn = [None] * 32

            def emit_proj_chunk(sb8):
                """fp8 DoubleRow projections for one 512-column x chunk."""
                scols = slice(sb8 * 512, (sb8 + 1) * 512)
                xts = {}
                for hl, xd in (("h", xh_d), ("l", xl_d)):
                    xt = xt_pool.tile([128, 8, 512], FP8, tag=f"x{hl}",
                                      name=f"x{hl}{sb8}")
                    for kt2 in range(4):
                        nc.sync.dma_start(
                            out=xt[:, 2 * kt2:2 * kt2 + 2, :],
                            in_=xd[:, scols].rearrange("(kt p) s -> p kt s", kt=8)
                            [:, 2 * kt2:2 * kt2 + 2, :])
                    xts[hl] = xt
                for name in ("q", "k", "v"):
                    wh, wl = wsb[name]
                    psp = ps_x.tile([128, 512], F32, tag="ps_x",
                                    name=f"psp{name}{sb8}")
                    terms = ((wh, xts["h"]), (wl, xts["h"]), (wh, xts["l"]))
                    i_mm = 0
                    for wt, xt in terms:
                        for kp in range(4):
                            nc.tensor.matmul(psp, wt[:, 2 * kp:2 * kp + 2, :],
                                             xt[:, 2 * kp:2 * kp + 2, :],
                                             start=(i_mm == 0), stop=(i_mm == 11),
                                             perf_mode=DR)
                            i_mm += 1
                    dst = {"q": qT, "k": kT, "v": vstk}[name]
                    nc.vector.tensor_scalar(out=dst[:, scols], in0=psp,
                                            scalar1=bsb[name], scalar2=None, op0=ALU.add)

            def emit_vdram_writes(b):
                bc = slice(b * S, (b + 1) * S)
                vdw_b[b] = [
                    nc.gpsimd.dma_start(out=vdram[0:64, bc], in_=vstk[0:64, bc]),
                    nc.gpsimd.dma_start(out=vdram[65:129, bc], in_=vstk[64:128, bc]),
                ]

            def emit_vn_transposes(b):
                for jt in range(b * 16, b * 16 + 16):
                    t = vn_pool.tile([128, 144], BF16, tag="vn", name=f"vn{jt}")
                    rd = nc.sync.dma_start(out=t,
                                           in_=vdram[:, jt * 128:(jt + 1) * 128],
                                           transpose=True)
                    rd = rd.ins if hasattr(rd, "ins") else rd
                    for w in vdw_const + vdw_b[b]:
                        add_dep_helper(rd, w.ins if hasattr(w, "ins") else w,
                                       reason="vdram write before transpose read")
                    vn[jt] = t

            # ---- batch-0 projection: first 2 chunks only, rest injected ----
            for sb8 in range(2):
                emit_proj_chunk(sb8)

            tap_d = {}
            if taps:
                for nm, shp, dt_ in (("qT", [128, BS], BF16), ("kT", [128, BS], BF16),
                                     ("vstk", [128, BS], BF16)):
                    tap_d[nm] = nc.declare_dram_parameter(f"tap_{nm}", shp, dt_,
                                                          isOutput=True)

            # ---- attention phases, software-pipelined across boundaries ----
            PH = [(0, 0), (0, 1), (1, 0), (1, 1)]

            def mkphase(pi):
                b, ih = PH[pi]
                return {"pi": pi, "b": b, "ih": ih, "i0": b * S + ih * 1024,
                        "et": {}, "pso0": {}, "vrec": {}, "pv_ptr": 0}

            def emit_pv(ph, pso, h, i2, jt, start, stop):
                nc.tensor.matmul(pso, vn[ph["b"] * 16 + jt][:, h * 65:h * 65 + 65],
                                 ph["et"][(h, jt)][:, i2 * 512:(i2 + 1) * 512],
                                 start=start, stop=stop)

            def emit_scores_jt(ph, jt):
                b, ih, i0 = ph["b"], ph["ih"], ph["i0"]
                jcol = slice(b * S + jt * 128, b * S + (jt + 1) * 128)
                with tc.high_priority():
                    psA = ps_scA.tile([128, 1024], F32, tag="scA")
                    psB = ps_scB.tile([128, 1024], F32, tag="scB")
                    for h, (pst, tp) in ((0, (psA, (0, 0))), (1, (psB, (64, 0)))):
                        hp = slice(h * 64, (h + 1) * 64)
                        for s2 in range(2):
                            icols = slice(i0 + s2 * 512, i0 + (s2 + 1) * 512)
                            nc.tensor.matmul(pst[:, s2 * 512:(s2 + 1) * 512],
                                             kT[hp, jcol], qT[hp, icols],
                                             start=True, stop=True, tile_position=tp)
                    for h, pst in ((0, psA), (1, psB)):
                        e_t = et_pool.tile([128, 1024], BF16, tag="et",
                                           name=f"et{b}{ih}{h}{jt}")
                        nc.scalar.activation(e_t, pst, AF.Exp,
                                             bias=0.0, scale=EXP_SCALE)
                        ph["et"][(h, jt)] = e_t

            def emit_vvrec(ph):
                icols_all = slice(ph["i0"], ph["i0"] + 1024)
                tvv = sb_x.tile([128, 1024], BF16, tag="tvv")
                nc.vector.tensor_tensor(out=tvv, in0=vstk[:, icols_all],
                                        in1=vstk[:, icols_all], op=ALU.mult)
                for s2 in range(2):
                    ps_vv = ps_x.tile([33, 512], F32, tag="ps_x")
                    nc.tensor.matmul(ps_vv, vvw, tvv[:, s2 * 512:(s2 + 1) * 512],
                                     start=True, stop=True)
                    for h in range(2):
                        veps = sb_s.tile([1, 512], F32, tag="veps", bufs=2)
                        nc.vector.tensor_scalar(out=veps,
                                                in0=ps_vv[32 * h:32 * h + 1, :],
                                                scalar1=EPS_SCALED, scalar2=None,
                                                op0=ALU.add)
                        vr = sb_s.tile([1, 512], F32, tag="vrec", bufs=6)
                        nc.vector.reciprocal(vr, veps)
                        ph["vrec"][(h, s2)] = vr

            def emit_exclusion(ph, pso, h, i2):
                b, ih, i0 = ph["b"], ph["ih"], ph["i0"]
                hp = slice(h * 64, (h + 1) * 64)
                vcols2 = slice(i0 + i2 * 512, i0 + (i2 + 1) * 512)
                tov = sb_x.tile([64, 512], BF16, tag="tov")
                nc.vector.tensor_tensor(out=tov, in0=pso[0:64, :],
                                        in1=vstk[hp, vcols2], op=ALU.mult)
                ps_ov = ps_x.tile([1, 512], F32, tag="ps_x")
                nc.tensor.matmul(ps_ov, ones64, tov, start=True, stop=True)
                r_t = sb_s.tile([1, 512], BF16, tag="r_t", bufs=6)
                with nc.allow_low_precision(reason="softmax scale bf16 by design"):
                    nc.vector.reciprocal(r_t, pso[64:65, :])
                align = sb_s.tile([1, 512], BF16, tag="align", bufs=6)
                nc.vector.tensor_tensor(out=align, in0=ps_ov,
                                        in1=ph["vrec"][(h, i2)], op=ALU.mult)
                ps_bc = ps_x.tile([128, 512], F32, tag="ps_x")
                nc.tensor.matmul(ps_bc[0:64, :], ones1, r_t,
                                 start=True, stop=True, tile_position=(0, 0))
                nc.tensor.matmul(ps_bc[64:128, :], ones1, align,
                                 start=True, stop=True, tile_position=(0, 64))
                t2 = sb_x.tile([64, 512], F32, tag="t2")
                nc.vector.tensor_tensor(out=t2, in0=ps_bc[64:128, :],
                                        in1=vstk[hp, vcols2], op=ALU.mult)
                t3 = sb_x.tile([64, 512], F32, tag="t3")
                nc.vector.tensor_tensor(out=t3, in0=pso[0:64, :],
                                        in1=t2, op=ALU.subtract)
                nc.vector.tensor_tensor(
                    out=o_fT[(b, ih)][h * 64:(h + 1) * 64,
                                      i2 * 512:(i2 + 1) * 512],
                    in0=ps_bc[0:64, :], in1=t3, op=ALU.mult)

            def emit_outproj(ph, st8s):
                b, ih = ph["b"], ph["ih"]
                for st8 in st8s:
                    st = 8 * ih + st8
                    for eb in range(2):
                        ps_o2 = ps_x.tile([128, 512], F32, tag="ps_x")
                        nc.tensor.matmul(ps_o2,
                                         o_fT[(b, ih)][:, st8 * 128:(st8 + 1) * 128],
                                         wo_sb[:, eb * 512:(eb + 1) * 512],
                                         start=True, stop=True)
                        stg = out_stage.tile([128, 512], BF16, tag="ostg")
                        if ph["pi"] == 3 and (st8 + eb) % 2 == 0:
                            nc.scalar.copy(stg, ps_o2)
                        else:
                            nc.vector.tensor_copy(stg, ps_o2)
                        nc.sync.dma_start(
                            out=part_d[b * S + st * 128:b * S + (st + 1) * 128,
                                       eb * 512:(eb + 1) * 512],
                            in_=stg)

            def pv_backlog(ph, jt, limit):
                n = 0
                while ph["pv_ptr"] < jt and n < limit:
                    j = ph["pv_ptr"]
                    for h in range(2):
                        if j == 0:
                            ph["pso0"][h] = ps_pv.tile(
                                [65, 512], F32, tag="pv",
                                name=f"pv{ph['b']}{ph['ih']}{h}0")
                        emit_pv(ph, ph["pso0"][h], h, 0, j,
                                start=(j == 0), stop=False)
                    ph["pv_ptr"] = j + 1
                    n += 1

            # phase 0 injections: proj chunks, vdram writes, vn transposes
            def ph0_inject(jt):
                if jt == 3:
                    emit_proj_chunk(2)
                elif jt == 4:
                    emit_proj_chunk(3)
                elif jt == 5:
                    emit_vdram_writes(0)
                    emit_vn_transposes(0)
                    for wc in range(4):
                        nc.sync.dma_start(out=wo_sb[:, wc * 256:(wc + 1) * 256],
                                          in_=woT_d[:, wc * 256:(wc + 1) * 256])
                elif jt in (7, 10):
                    emit_proj_chunk(4 + (jt - 7) // 3)

            def phase_core(ph, start_jt):
                pi = ph["pi"]
                for jt in range(start_jt, 16):
                    emit_scores_jt(ph, jt)
                    if pi == 0:
                        ph0_inject(jt)
                        if jt >= 8:
                            pv_backlog(ph, jt, 16)
                    else:
                        if pi == 1:
                            if jt in (4, 7):
                                emit_proj_chunk(6 + (jt - 4) // 3)
                            elif jt == 8:
                                emit_vdram_writes(1)
                                emit_vn_transposes(1)
                        pv_backlog(ph, jt, 16)

            def phase_post(ph):
                def a0():
                    for h in range(2):
                        emit_pv(ph, ph["pso0"][h], h, 0, 15, start=False, stop=True)
                def a1():
                    emit_exclusion(ph, ph["pso0"][0], 0, 0)
                def a2():
                    emit_exclusion(ph, ph["pso0"][1], 1, 0)
                def c():
                    if ph["pi"] == 3:
                        big = ps_scA.tile([128, 1024], F32, tag="scA",
                                          name=f"pvz{ph['b']}{ph['ih']}0")
                        pso1 = big[0:65, 0:512]
                    else:
                        pso1 = ps_pv.tile([65, 512], F32, tag="pv",
                                          name=f"pv{ph['b']}{ph['ih']}01")
                    for jt in range(16):
                        emit_pv(ph, pso1, 0, 1, jt, start=(jt == 0), stop=(jt == 15))
                    emit_exclusion(ph, pso1, 0, 1)
                def d():
                    if ph["pi"] == 3:
                        big = ps_scB.tile([128, 1024], F32, tag="scB",
                                          name=f"pvz{ph['b']}{ph['ih']}1")
                        pso1 = big[0:65, 0:512]
                    else:
                        pso1 = ps_pv.tile([65, 512], F32, tag="pv",
                                          name=f"pv{ph['b']}{ph['ih']}11")
                    for jt in range(16):
                        emit_pv(ph, pso1, 1, 1, jt, start=(jt == 0), stop=(jt == 15))
                    emit_exclusion(ph, pso1, 1, 1)
                def bthunk():
                    emit_outproj(ph, range(0, 4))
                def e():
                    emit_outproj(ph, range(4, 8))
                return [a0, a1, a2, c, d, bthunk, e]

            ph = mkphase(0)
            emit_vvrec(ph)
            for jt in range(3):
                emit_scores_jt(ph, jt)
            phase_core(ph, 3)
            posts = phase_post(ph)
            for pi in range(1, 4):
                nxt = mkphase(pi)
                pres = [lambda: (emit_scores_jt(nxt, 0), emit_vvrec(nxt)),
                        lambda: emit_scores_jt(nxt, 1),
                        lambda: emit_scores_jt(nxt, 2),
                        lambda: emit_scores_jt(nxt, 3)]
                seq = [posts[0], posts[1], posts[2], pres[0], posts[3], pres[1],
                       posts[4], pres[2], posts[5], pres[3], posts[6]]
                for t in seq:
                    t()
                phase_core(nxt, 4)
                posts = phase_post(nxt)
                ph = nxt
            for t in posts:
                t()

            if taps:
                nc.sync.dma_start(out=tap_d["qT"][:, :], in_=qT)
                nc.sync.dma_start(out=tap_d["kT"][:, :], in_=kT)
                nc.sync.dma_start(out=tap_d["vstk"][:, :], in_=vstk)

    split_waits(nc)
    return nc


_CACHE = {}
F8NP = ml_dtypes.float8_e4m3fn


def make_in_maps(x, wq, bq, wk, bk, wv, bv, wo):
    x = np.ascontiguousarray(np.asarray(x, dtype=np.float32))
    xT = np.ascontiguousarray(x.reshape(BS, D).T)
    xh = xT.astype(F8NP)
    xl = (xT - xh.astype(np.float32)).astype(F8NP)
    in_maps = []
    for g in range(N_CORES):
        cs = slice(g * E_LOC, (g + 1) * E_LOC)
        m = {"xh": xh, "xl": xl}
        for name, w, bias in (("q", wq, bq), ("k", wk, bk), ("v", wv, bv)):
            wT = np.ascontiguousarray(w[cs, :].T.astype(np.float32)) * W_SCALE
            wh = wT.astype(F8NP)
            wl = (wT - wh.astype(np.float32)).astype(F8NP)
            m[f"w{name}h"] = wh
            m[f"w{name}l"] = wl
            m[f"b{name}"] = np.ascontiguousarray(bias[cs].astype(np.float32)) * W_SCALE
        m["woT"] = np.ascontiguousarray(wo[:, cs].T.astype(np.float32)).astype(
            ml_dtypes.bfloat16)
        in_maps.append(m)
    return in_maps


def kernel(x, wq, bq, wk, bk, wv, bv, wo, bo):
    wq, wk, wv, wo = (np.asarray(w, dtype=np.float32) for w in (wq, wk, wv, wo))
    bq, bk, bv, bo = (np.asarray(v, dtype=np.float32) for v in (bq, bk, bv, bo))

    if "nc" not in _CACHE:
        _CACHE["nc"] = build_program()
    nc = _CACHE["nc"]

    in_maps = make_in_maps(x, wq, bq, wk, bk, wv, bv, wo)
    res = run_bass_kernel_spmd(nc, in_maps, list(range(N_CORES)))
    out = np.zeros((BS, D), np.float32)
    for g in range(N_CORES):
        out += np.asarray(res.results[g]["partial"], np.float32)
    out = out / W_SCALE + bo[None, :]
    return out.reshape(B, S, D).astype(np.float32)


# revision 51
# speedup vs baseline: 1.0131x; 1.0131x over previous
"""ExclusiveSelfAttention TRN2 kernel: head-sharded tensor parallel over 8 NeuronCores.

Sharding: 16 heads / 8 cores = 2 heads (128 channels) per core.
Each core computes q/k/v projections for its 2 heads (full sequence),
attention + per-position Gram-Schmidt exclusion (head-local), and a
partial output projection (contraction over its 128 channels).
The host sums the 8 partials, divides by 32 and adds the output bias.

Precision: projections run as fp8e4m3 DoubleRow matmuls with 3 residual
terms (w_hi x_hi + w_lo x_hi + w_hi x_lo), w pre-scaled by 32 on the host
so w and its fp8 residual stay in fp8's dynamic range; the 32x scale rides
through scores (folded into the exp scale) and v (cancels in softmax
normalization / exclusion) and is divided out on the host. Everything
else is bf16 with fp32 PSUM accumulation (fp8 attention was tried and
fails the 2e-2 gate: fp8's 3-bit mantissa on softmax weights alone costs
~2e-2). Measured rel err ~6.8e-3.

Schedule: batch-0 projection first; batch-1 projection chunks are
injected into phase (0,0)'s jt loop, which is exp(ACT)-paced and leaves
PE idle slots. PV for i2=0 runs inside the jt loop one j-tile behind
scores; i2=1 PV, exclusion, and the output projection trail each phase
and overlap the next phase's scores.
"""

import sys

if '/opt/trn_rl_repo' not in sys.path:
    sys.path.insert(0, '/opt/trn_rl_repo')

import numpy as np
import ml_dtypes

import concourse.bass as bass
import concourse.mybir as mybir
import concourse.tile as tile
from concourse.bass_utils import run_bass_kernel_spmd

F32 = mybir.dt.float32
BF16 = mybir.dt.bfloat16
FP8 = mybir.dt.float8e4
AF = mybir.ActivationFunctionType
ALU = mybir.AluOpType
DR = mybir.MatmulPerfMode.DoubleRow

B, S, D = 2, 2048, 1024
BS = B * S                    # 4096 combined (b, s) rows
HD = 64                       # head dim
E_LOC = 128                   # channels per core (2 heads)
N_CORES = 8
W_SCALE = 32.0                # host multiplies w (and biases) by this
EPS_SCALED = 1e-8 * W_SCALE * W_SCALE
EXP_SCALE = 0.125 / (W_SCALE * W_SCALE)    # 1/8 true scale on 1024x scores

_ENGINE_TO_NC = {"PE": "tensor", "DVE": "vector", "Activation": "scalar",
                 "Pool": "gpsimd", "SP": "sync"}


def _make_nop(nc, engine):
    eng = getattr(nc, _ENGINE_TO_NC[str(engine).split(".")[-1]])
    r = eng.nop(nofuse=True, hint="waitsplit")
    ins = r.ins if hasattr(r, "ins") else r
    for blk in nc.main_func.blocks:
        insns = blk.instructions
        for i, x in enumerate(insns):
            if x.name == ins.name:
                del insns[i]
                blk.instructions = insns
                return ins
    raise RuntimeError("freshly created nop not found")


def split_waits(nc, limit=1):
    """Walrus codegen only encodes one sync-wait per instruction here; move
    excess waits onto preceding same-engine NOPs (same-engine program order
    makes this semantics-preserving)."""
    for blk in nc.main_func.blocks:
        ins_list = blk.instructions
        out, changed = [], False
        for ins in ins_list:
            si = ins.sync_info
            if si is not None and len(si.on_wait) > limit:
                waits = list(si.on_wait)
                extra, keep = waits[:-limit], waits[-limit:]
                for w in extra:
                    nop = _make_nop(nc, ins.engine)
                    nop.sync_info = mybir.SyncInfo(on_wait=[w], on_update=[])
                    out.append(nop)
                ins.sync_info = mybir.SyncInfo(on_wait=keep, on_update=list(si.on_update))
                changed = True
            out.append(ins)
        if changed:
            blk.instructions = out


def build_program():
    import os
    taps = os.environ.get("KTAPS") == "1"
    nc = bass.Bass()

    xh_d = nc.declare_dram_parameter("xh", [D, BS], FP8, isOutput=False)
    xl_d = nc.declare_dram_parameter("xl", [D, BS], FP8, isOutput=False)
    w_d = {}
    for name in ("q", "k", "v"):
        w_d[name] = (nc.declare_dram_parameter(f"w{name}h", [D, E_LOC], FP8, isOutput=False),
                     nc.declare_dram_parameter(f"w{name}l", [D, E_LOC], FP8, isOutput=False))
    bq_d = nc.declare_dram_parameter("bq", [E_LOC], F32, isOutput=False)
    bk_d = nc.declare_dram_parameter("bk", [E_LOC], F32, isOutput=False)
    bv_d = nc.declare_dram_parameter("bv", [E_LOC], F32, isOutput=False)
    woT_d = nc.declare_dram_parameter("woT", [E_LOC, D], BF16, isOutput=False)
    part_d = nc.declare_dram_parameter("partial", [BS, D], BF16, isOutput=True)

    with tile.TileContext(nc) as tc:
        import contextlib
        with contextlib.ExitStack() as ctx:
            const = ctx.enter_context(tc.tile_pool(name="const", bufs=1))
            xt_pool = ctx.enter_context(tc.tile_pool(name="xt", bufs=2))
            persist = ctx.enter_context(tc.tile_pool(name="persist", bufs=1))
            et_pool = ctx.enter_context(tc.tile_pool(name="et", bufs=46))
            vn_pool = ctx.enter_context(tc.tile_pool(name="vn", bufs=32))
            sb_x = ctx.enter_context(tc.tile_pool(name="sb_x", bufs=2))
            sb_s = ctx.enter_context(tc.tile_pool(name="sb_s", bufs=2))
            out_stage = ctx.enter_context(tc.tile_pool(name="ostg", bufs=8))
            dram = ctx.enter_context(tc.tile_pool(name="dram", bufs=1, space="DRAM"))
            ps_scA = ctx.enter_context(tc.tile_pool(name="ps_scA", bufs=1, space="PSUM"))
            ps_scB = ctx.enter_context(tc.tile_pool(name="ps_scB", bufs=1, space="PSUM"))
            ps_pv = ctx.enter_context(tc.tile_pool(name="ps_pv", bufs=2, space="PSUM"))
            ps_x = ctx.enter_context(tc.tile_pool(name="ps_x", bufs=2, space="PSUM"))

            # ---- constants / weights ----
            wsb = {}
            for name in ("q", "k", "v"):
                pair = []
                for hl, wd in zip("hl", w_d[name]):
                    t = const.tile([128, 8, E_LOC], FP8, tag=f"w{name}{hl}")
                    nc.sync.dma_start(out=t, in_=wd[:, :].rearrange(
                        "(kt p) e -> p kt e", kt=8))
                    pair.append(t)
                wsb[name] = pair
            bsb = {}
            for name, bd in (("q", bq_d), ("k", bk_d), ("v", bv_d)):
                t = const.tile([128, 1], F32, tag=f"b{name}")
                nc.sync.dma_start(out=t, in_=bd[:].rearrange("(p one) -> p one", one=1))
                bsb[name] = t
            ones64 = const.tile([64, 1], BF16, tag="ones64")
            nc.vector.memset(ones64, 1.0)
            # vv reducer: h0 sum -> out partition 0, h1 sum -> partition 32
            vvw = const.tile([128, 33], BF16, tag="vvw")
            nc.vector.memset(vvw, 0.0)
            nc.vector.memset(vvw[0:64, 0:1], 1.0)
            nc.vector.memset(vvw[64:128, 32:33], 1.0)
            ones1 = const.tile([1, 64], BF16, tag="ones1")
            nc.vector.memset(ones1, 1.0)
            ones_row = const.tile([128, 32], BF16, tag="ones_row")
            nc.vector.memset(ones_row, 1.0)
            wo_sb = const.tile([128, D], BF16, tag="wo")

            # ---- persistent activations ----
            qT = persist.tile([128, BS], BF16, tag="qT")       # [e_loc, b*s]
            kT = persist.tile([128, BS], BF16, tag="kT")
            vstk = persist.tile([128, BS], BF16, tag="vstk")   # stacked heads, 32x
            o_fT = {(b, ih): persist.tile([128, 1024], BF16, tag=f"ofT{b}{ih}",
                                          name=f"ofT{b}{ih}")
                    for b in range(B) for ih in range(2)}

            from concourse.tile import add_dep_helper
            vdram = dram.tile([144, BS], BF16, tag="vdram")

            def _row_ap(r):
                return vdram[r:r + 1, :].rearrange("one (p f) -> (one p) f", p=128)

            vdw_const = [nc.gpsimd.dma_start(out=_row_ap(64), in_=ones_row),
                         nc.gpsimd.dma_start(out=_row_ap(129), in_=ones_row)]
            vdw_const += [nc.gpsimd.dma_start(out=_row_ap(130 + pr), in_=ones_row)
                          for pr in range(14)]
            vdw_b = {}
            vn = [None] * 32

            def emit_proj_chunk(sb8):
                """fp8 DoubleRow projections for one 512-column x chunk."""
                scols = slice(sb8 * 512, (sb8 + 1) * 512)
                xts = {}
                for hl, xd in (("h", xh_d), ("l", xl_d)):
                    xt = xt_pool.tile([128, 8, 512], FP8, tag=f"x{hl}",
                                      name=f"x{hl}{sb8}")
                    for kt2 in range(4):
                        nc.sync.dma_start(
                            out=xt[:, 2 * kt2:2 * kt2 + 2, :],
                            in_=xd[:, scols].rearrange("(kt p) s -> p kt s", kt=8)
                            [:, 2 * kt2:2 * kt2 + 2, :])
                    xts[hl] = xt
                for name in ("q", "k", "v"):
                    wh, wl = wsb[name]
                    psp = ps_x.tile([128, 512], F32, tag="ps_x",
                                    name=f"psp{name}{sb8}")
                    terms = ((wh, xts["h"]), (wl, xts["h"]), (wh, xts["l"]))
                    i_mm = 0
                    for wt, xt in terms:
                        for kp in range(4):
                            nc.tensor.matmul(psp, wt[:, 2 * kp:2 * kp + 2, :],
                                             xt[:, 2 * kp:2 * kp + 2, :],
                                             start=(i_mm == 0), stop=(i_mm == 11),
                                             perf_mode=DR)
                            i_mm += 1
                    dst = {"q": qT, "k": kT, "v": vstk}[name]
                    nc.vector.tensor_scalar(out=dst[:, scols], in0=psp,
                                            scalar1=bsb[name], scalar2=None, op0=ALU.add)

            def emit_vdram_writes(b):
                bc = slice(b * S, (b + 1) * S)
                vdw_b[b] = [
                    nc.gpsimd.dma_start(out=vdram[0:64, bc], in_=vstk[0:64, bc]),
                    nc.gpsimd.dma_start(out=vdram[65:129, bc], in_=vstk[64:128, bc]),
                ]

            def emit_vn_transposes(b):
                for jt in range(b * 16, b * 16 + 16):
                    t = vn_pool.tile([128, 144], BF16, tag="vn", name=f"vn{jt}")
                    rd = nc.sync.dma_start(out=t,
                                           in_=vdram[:, jt * 128:(jt + 1) * 128],
                                           transpose=True)
                    rd = rd.ins if hasattr(rd, "ins") else rd
                    for w in vdw_const + vdw_b[b]:
                        add_dep_helper(rd, w.ins if hasattr(w, "ins") else w,
                                       reason="vdram write before transpose read")
                    vn[jt] = t

            # ---- batch-0 projection: first 2 chunks only, rest injected ----
            for sb8 in range(2):
                emit_proj_chunk(sb8)

            tap_d = {}
            if taps:
                for nm, shp, dt_ in (("qT", [128, BS], BF16), ("kT", [128, BS], BF16),
                                     ("vstk", [128, BS], BF16)):
                    tap_d[nm] = nc.declare_dram_parameter(f"tap_{nm}", shp, dt_,
                                                          isOutput=True)

            # ---- attention phases, software-pipelined across boundaries ----
            PH = [(0, 0), (0, 1), (1, 0), (1, 1)]

            def mkphase(pi):
                b, ih = PH[pi]
                return {"pi": pi, "b": b, "ih": ih, "i0": b * S + ih * 1024,
                        "et": {}, "pso0": {}, "vrec": {}, "pv_ptr": 0}

            def emit_pv(ph, pso, h, i2, jt, start, stop):
                nc.tensor.matmul(pso, vn[ph["b"] * 16 + jt][:, h * 65:h * 65 + 65],
                                 ph["et"][(h, jt)][:, i2 * 512:(i2 + 1) * 512],
                                 start=start, stop=stop)

            def emit_scores_jt(ph, jt):
                b, ih, i0 = ph["b"], ph["ih"], ph["i0"]
                jcol = slice(b * S + jt * 128, b * S + (jt + 1) * 128)
                with tc.high_priority():
                    psA = ps_scA.tile([128, 1024], F32, tag="scA")
                    psB = ps_scB.tile([128, 1024], F32, tag="scB")
                    for h, (pst, tp) in ((0, (psA, (0, 0))), (1, (psB, (64, 0)))):
                        hp = slice(h * 64, (h + 1) * 64)
                        for s2 in range(2):
                            icols = slice(i0 + s2 * 512, i0 + (s2 + 1) * 512)
                            nc.tensor.matmul(pst[:, s2 * 512:(s2 + 1) * 512],
                                             kT[hp, jcol], qT[hp, icols],
                                             start=True, stop=True, tile_position=tp)
                    for h, pst in ((0, psA), (1, psB)):
                        e_t = et_pool.tile([128, 1024], BF16, tag="et",
                                           name=f"et{b}{ih}{h}{jt}")
                        nc.scalar.activation(e_t, pst, AF.Exp,
                                             bias=0.0, scale=EXP_SCALE)
                        ph["et"][(h, jt)] = e_t

            def emit_vvrec(ph):
                icols_all = slice(ph["i0"], ph["i0"] + 1024)
                tvv = sb_x.tile([128, 1024], BF16, tag="tvv")
                nc.vector.tensor_tensor(out=tvv, in0=vstk[:, icols_all],
                                        in1=vstk[:, icols_all], op=ALU.mult)
                for s2 in range(2):
                    ps_vv = ps_x.tile([33, 512], F32, tag="ps_x")
                    nc.tensor.matmul(ps_vv, vvw, tvv[:, s2 * 512:(s2 + 1) * 512],
                                     start=True, stop=True)
                    veps = sb_s.tile([33, 512], F32, tag="veps", bufs=2)
                    nc.vector.tensor_scalar(out=veps, in0=ps_vv,
                                            scalar1=EPS_SCALED, scalar2=None,
                                            op0=ALU.add)
                    for h in range(2):
                        vr = sb_s.tile([1, 512], F32, tag="vrec", bufs=6)
                        nc.vector.reciprocal(vr, veps[32 * h:32 * h + 1, :])
                        ph["vrec"][(h, s2)] = vr

            def emit_exclusion(ph, pso, h, i2):
                b, ih, i0 = ph["b"], ph["ih"], ph["i0"]
                hp = slice(h * 64, (h + 1) * 64)
                vcols2 = slice(i0 + i2 * 512, i0 + (i2 + 1) * 512)
                tov = sb_x.tile([64, 512], BF16, tag="tov")
                nc.vector.tensor_tensor(out=tov, in0=pso[0:64, :],
                                        in1=vstk[hp, vcols2], op=ALU.mult)
                ps_ov = ps_x.tile([1, 512], F32, tag="ps_x")
                nc.tensor.matmul(ps_ov, ones64, tov, start=True, stop=True)
                r_t = sb_s.tile([1, 512], BF16, tag="r_t", bufs=6)
                with nc.allow_low_precision(reason="softmax scale bf16 by design"):
                    nc.vector.reciprocal(r_t, pso[64:65, :])
                align = sb_s.tile([1, 512], BF16, tag="align", bufs=6)
                nc.vector.tensor_tensor(out=align, in0=ps_ov,
                                        in1=ph["vrec"][(h, i2)], op=ALU.mult)
                ps_bc = ps_x.tile([128, 512], F32, tag="ps_x")
                nc.tensor.matmul(ps_bc[0:64, :], ones1, r_t,
                                 start=True, stop=True, tile_position=(0, 0))
                nc.tensor.matmul(ps_bc[64:128, :], ones1, align,
                                 start=True, stop=True, tile_position=(0, 64))
                t2 = sb_x.tile([64, 512], F32, tag="t2")
                nc.vector.tensor_tensor(out=t2, in0=ps_bc[64:128, :],
                                        in1=vstk[hp, vcols2], op=ALU.mult)
                t3 = sb_x.tile([64, 512], F32, tag="t3")
                nc.vector.tensor_tensor(out=t3, in0=pso[0:64, :],
                                        in1=t2, op=ALU.subtract)
                nc.vector.tensor_tensor(
                    out=o_fT[(b, ih)][h * 64:(h + 1) * 64,
                                      i2 * 512:(i2 + 1) * 512],
                    in0=ps_bc[0:64, :], in1=t3, op=ALU.mult)

            def emit_outproj(ph, st8s):
                b, ih = ph["b"], ph["ih"]
                for st8 in st8s:
                    st = 8 * ih + st8
                    for eb in range(2):
                        ps_o2 = ps_x.tile([128, 512], F32, tag="ps_x")
                        nc.tensor.matmul(ps_o2,
                                         o_fT[(b, ih)][:, st8 * 128:(st8 + 1) * 128],
                                         wo_sb[:, eb * 512:(eb + 1) * 512],
                                         start=True, stop=True)
                        stg = out_stage.tile([128, 512], BF16, tag="ostg")
                        if ph["pi"] == 3 and (st8 + eb) % 2 == 0:
                            nc.scalar.copy(stg, ps_o2)
                        else:
                            nc.vector.tensor_copy(stg, ps_o2)
                        nc.sync.dma_start(
                            out=part_d[b * S + st * 128:b * S + (st + 1) * 128,
                                       eb * 512:(eb + 1) * 512],
                            in_=stg)

            def pv_backlog(ph, jt, limit):
                n = 0
                while ph["pv_ptr"] < jt and n < limit:
                    j = ph["pv_ptr"]
                    for h in range(2):
                        if j == 0:
                            ph["pso0"][h] = ps_pv.tile(
                                [65, 512], F32, tag="pv",
                                name=f"pv{ph['b']}{ph['ih']}{h}0")
                        emit_pv(ph, ph["pso0"][h], h, 0, j,
                                start=(j == 0), stop=False)
                    ph["pv_ptr"] = j + 1
                    n += 1

            # phase 0 injections: proj chunks, vdram writes, vn transposes
            def ph0_inject(jt):
                if jt == 3:
                    emit_proj_chunk(2)
                elif jt == 4:
                    emit_proj_chunk(3)
                elif jt == 5:
                    emit_vdram_writes(0)
                    emit_vn_transposes(0)
                    for wc in range(4):
                        nc.sync.dma_start(out=wo_sb[:, wc * 256:(wc + 1) * 256],
                                          in_=woT_d[:, wc * 256:(wc + 1) * 256])
                elif jt in (7, 10):
                    emit_proj_chunk(4 + (jt - 7) // 3)

            def phase_core(ph, start_jt):
                pi = ph["pi"]
                for jt in range(start_jt, 16):
                    emit_scores_jt(ph, jt)
                    if pi == 0:
                        ph0_inject(jt)
                        if jt >= 8:
                            pv_backlog(ph, jt, 16)
                    else:
                        if pi == 1:
                            if jt in (4, 7):
                                emit_proj_chunk(6 + (jt - 4) // 3)
                            elif jt == 8:
                                emit_vdram_writes(1)
                                emit_vn_transposes(1)
                        pv_backlog(ph, jt, 16)

            def phase_post(ph):
                def a0():
                    for h in range(2):
                        emit_pv(ph, ph["pso0"][h], h, 0, 15, start=False, stop=True)
                def a1():
                    emit_exclusion(ph, ph["pso0"][0], 0, 0)
                def a2():
                    emit_exclusion(ph, ph["pso0"][1], 1, 0)
                def c():
                    if ph["pi"] == 3:
                        big = ps_scA.tile([128, 1024], F32, tag="scA",
                                          name=f"pvz{ph['b']}{ph['ih']}0")
                        pso1 = big[0:65, 0:512]
                    else:
                        pso1 = ps_pv.tile([65, 512], F32, tag="pv",
                                          name=f"pv{ph['b']}{ph['ih']}01")
                    for jt in range(16):
                        emit_pv(ph, pso1, 0, 1, jt, start=(jt == 0), stop=(jt == 15))
                    emit_exclusion(ph, pso1, 0, 1)
                def d():
                    if ph["pi"] == 3:
                        big = ps_scB.tile([128, 1024], F32, tag="scB",
                                          name=f"pvz{ph['b']}{ph['ih']}1")
                        pso1 = big[0:65, 0:512]
                    else:
                        pso1 = ps_pv.tile([65, 512], F32, tag="pv",
                                          name=f"pv{ph['b']}{ph['ih']}11")
                    for jt in range(16):
                        emit_pv(ph, pso1, 1, 1, jt, start=(jt == 0), stop=(jt == 15))
                    emit_exclusion(ph, pso1, 1, 1)
                def bthunk():
                    emit_outproj(ph, range(0, 4))
                def e():
                    emit_outproj(ph, range(4, 8))
                return [a0, a1, a2, c, d, bthunk, e]

            ph = mkphase(0)
            emit_vvrec(ph)
            for jt in range(3):
                emit_scores_jt(ph, jt)
            phase_core(ph, 3)
            posts = phase_post(ph)
            for pi in range(1, 4):
                nxt = mkphase(pi)
                pres = [lambda: (emit_scores_jt(nxt, 0), emit_vvrec(nxt)),
                        lambda: emit_scores_jt(nxt, 1),
                        lambda: emit_scores_jt(nxt, 2),
                        lambda: emit_scores_jt(nxt, 3)]
                seq = [posts[0], posts[1], posts[2], pres[0], posts[3], pres[1],
                       posts[4], pres[2], posts[5], pres[3], posts[6]]
                for t in seq:
                    t()
                phase_core(nxt, 4)
                posts = phase_post(nxt)
                ph = nxt
            for t in posts:
                t()

            if taps:
                nc.sync.dma_start(out=tap_d["qT"][:, :], in_=qT)
                nc.sync.dma_start(out=tap_d["kT"][:, :], in_=kT)
                nc.sync.dma_start(out=tap_d["vstk"][:, :], in_=vstk)

    split_waits(nc)
    return nc


_CACHE = {}
F8NP = ml_dtypes.float8_e4m3fn


def make_in_maps(x, wq, bq, wk, bk, wv, bv, wo):
    x = np.ascontiguousarray(np.asarray(x, dtype=np.float32))
    xT = np.ascontiguousarray(x.reshape(BS, D).T)
    xh = xT.astype(F8NP)
    xl = (xT - xh.astype(np.float32)).astype(F8NP)
    in_maps = []
    for g in range(N_CORES):
        cs = slice(g * E_LOC, (g + 1) * E_LOC)
        m = {"xh": xh, "xl": xl}
        for name, w, bias in (("q", wq, bq), ("k", wk, bk), ("v", wv, bv)):
            wT = np.ascontiguousarray(w[cs, :].T.astype(np.float32)) * W_SCALE
            wh = wT.astype(F8NP)
            wl = (wT - wh.astype(np.float32)).astype(F8NP)
            m[f"w{name}h"] = wh
            m[f"w{name}l"] = wl
            m[f"b{name}"] = np.ascontiguousarray(bias[cs].astype(np.float32)) * W_SCALE
        m["woT"] = np.ascontiguousarray(wo[:, cs].T.astype(np.float32)).astype(
            ml_dtypes.bfloat16)
        in_maps.append(m)
    return in_maps


def kernel(x, wq, bq, wk, bk, wv, bv, wo, bo):
    wq, wk, wv, wo = (np.asarray(w, dtype=np.float32) for w in (wq, wk, wv, wo))
    bq, bk, bv, bo = (np.asarray(v, dtype=np.float32) for v in (bq, bk, bv, bo))

    if "nc" not in _CACHE:
        _CACHE["nc"] = build_program()
    nc = _CACHE["nc"]

    in_maps = make_in_maps(x, wq, bq, wk, bk, wv, bv, wo)
    res = run_bass_kernel_spmd(nc, in_maps, list(range(N_CORES)))
    out = np.zeros((BS, D), np.float32)
    for g in range(N_CORES):
        out += np.asarray(res.results[g]["partial"], np.float32)
    out = out / W_SCALE + bo[None, :]
    return out.reshape(B, S, D).astype(np.float32)
